# revision 4
# baseline (speedup 1.0000x reference)
"""V2: folded direct matmul using the Makhoul reflection symmetry.

For A = build_A(expk):  A[k, N-1-n] = (-1)^k A[k, n]  (holds for arbitrary
expk: it follows from the even/odd permutation structure).  So both stage
contractions fold to length N/2:
  stage1: Mfold[b, k] = sum_{a<H} A1[k, a] * xq[a + H*par(k), b]
  stage2: out[l, k]   = sum_{rf<H} A0[l, rf] * Mfold[b = rf + H*par(l), k]
where xq is x^T folded on both axes (sum-half / diff-half) on the host, and
Mfold rows b<H are the r-sum-folded stage-1 results (b>=H: diff).

Per core: 512 output columns.  PE work per core: 2 * 4.3e9 MACs.
"""
import numpy as np

N = 4096
H = N // 2
P = 128
NT = N // P
KC = 512
NCORES = 8

_NC_CACHE = {}
CHAIN_NAME = "a1w"


def _makhoul_perm(n):
    j = np.arange(n)
    return np.where(j < n // 2, 2 * j, 2 * (n - 1 - j) + 1)


def _build_A(expk, n):
    c = expk[:, 0].astype(np.float64)
    s = expk[:, 1].astype(np.float64)
    k = np.arange(n, dtype=np.int64)
    j = np.arange(n, dtype=np.int64)
    ang = (2.0 * np.pi / n) * ((k[:, None] * j[None, :]) % n).astype(np.float64)
    B = c[:, None] * np.cos(ang) + s[:, None] * np.sin(ang)
    A = np.empty((n, n), dtype=np.float64)
    A[:, _makhoul_perm(n)] = B
    return A.astype(np.float32)


def _fold_rows(m):
    """[N, ...] -> sum-half / diff-half stacked [N, ...]."""
    top, bot = m[:H], m[H:][::-1]
    return np.concatenate([top + bot, top - bot], axis=0)


def _prep(x, expk0, expk1):
    x = np.asarray(x, dtype=np.float32)
    A1 = _build_A(np.asarray(expk1, np.float32), N)
    A0 = _build_A(np.asarray(expk0, np.float32), N)
    xt = np.ascontiguousarray(x.T)                       # [n, r]
    xq = _fold_rows(_fold_rows(xt).T).T                  # fold n (rows) & r (cols)
    xq = np.ascontiguousarray(xq)

    # stage-2 stationary: a0w[:H, j] = A0[2j, rf], a0w[H:, j] = A0[2j+1, rf]
    a0w = np.empty((N, H), dtype=np.float32)
    a0w[:H] = A0[0::2, :H].T
    a0w[H:] = A0[1::2, :H].T
    a0w = np.ascontiguousarray(a0w)

    in_maps = []
    for c in range(NCORES):
        kc = slice(c * KC, (c + 1) * KC)
        A1c = A1[kc]                                     # [512, n]
        a1w = np.zeros((N, KC), dtype=np.float32)
        a1w[:H, :256] = A1c[0::2, :H].T                  # even-k weights
        a1w[H:, 256:] = A1c[1::2, :H].T                  # odd-k weights
        in_maps.append({"xq": xq, "a1w": np.ascontiguousarray(a1w), "a0w": a0w})
    return in_maps


def _host_sim(x, expk0, expk1):
    """Numpy simulation of the kernel dataflow (for validation)."""
    in_maps = _prep(x, expk0, expk1)
    outs = []
    for c in range(NCORES):
        m = in_maps[c]
        xq, a1w, a0w = m["xq"], m["a1w"], m["a0w"]
        mfold = np.empty((N, KC), dtype=np.float32)
        me = xq[:H].T @ a1w[:H, :256]                    # [b, 256] even k
        mo = xq[H:].T @ a1w[H:, 256:]                    # [b, 256] odd k
        mfold[:, 0::2] = me
        mfold[:, 1::2] = mo
        out = np.empty((N, KC), dtype=np.float32)
        out[0::2] = (a0w[:H].T @ mfold[:H])              # even l
        out[1::2] = (a0w[H:].T @ mfold[H:])              # odd l
        outs.append(out)
    return np.concatenate(outs, axis=1)


def _build_nc(reps=1):
    import concourse.bacc as bacc
    import concourse.mybir as mybir
    import concourse.tile as tile

    FP = mybir.dt.float32
    nc = bacc.Bacc("TRN2", target_bir_lowering=False, debug=False,
                   num_devices=NCORES)

    xq_d = nc.dram_tensor("xq", [N, N], FP, kind="ExternalInput")
    a1w_d = nc.dram_tensor("a1w", [N, KC], FP, kind="ExternalInput")
    a0w_d = nc.dram_tensor("a0w", [N, H], FP, kind="ExternalInput")
    out_d = nc.dram_tensor("out", [N, KC], FP, kind="ExternalOutput")

    NH = H // P  # 16 chunks per folded (2048) contraction

    with tile.TileContext(nc) as tc:
      for _rep in range(reps):
        with tc.tile_pool(name="mfold", bufs=1) as mpool:
            # Mfold [b within tile, (b_tile, i, parity)]
            mf = mpool.tile([P, NT, 256, 2], FP)

            with (
                tc.tile_pool(name="a1pool", bufs=1) as a1pool,
                tc.tile_pool(name="xpool", bufs=2) as xpool,
                tc.tile_pool(name="ps1", bufs=4, space="PSUM") as ps1,
            ):
                a1t = a1pool.tile([P, NT, KC], FP)
                nc.sync.dma_start(
                    a1t[:], a1w_d[:].rearrange("(c p) k -> p c k", p=P))

                for bt in range(NT):
                    xb = xpool.tile([P, NT, P], FP)
                    nc.sync.dma_start(
                        xb[:],
                        xq_d[:, bt * P:(bt + 1) * P].rearrange(
                            "(c p) b -> p c b", p=P))
                    pse = ps1.tile([P, 256], FP)
                    pso = ps1.tile([P, 256], FP)
                    for ac in range(NH):
                        nc.tensor.matmul(
                            pse[:], xb[:, ac, :], a1t[:, ac, 0:256],
                            start=(ac == 0), stop=(ac == NH - 1))
                    for ac in range(NH):
                        nc.tensor.matmul(
                            pso[:], xb[:, NH + ac, :], a1t[:, NH + ac, 256:512],
                            start=(ac == 0), stop=(ac == NH - 1))
                    nc.vector.tensor_copy(mf[:, bt, :, 0], pse[:])
                    nc.vector.tensor_copy(mf[:, bt, :, 1], pso[:])

            with (
                tc.tile_pool(name="a0pool", bufs=2) as a0pool,
                tc.tile_pool(name="opool", bufs=4) as opool,
                tc.tile_pool(name="ps2", bufs=8, space="PSUM") as ps2,
            ):
                for par in range(2):
                    for lt in range(H // P):  # 16 tiles of 128 j's
                        ab = a0pool.tile([P, NH, P], FP)
                        nc.sync.dma_start(
                            ab[:],
                            a0w_d[par * H:(par + 1) * H,
                                  lt * P:(lt + 1) * P].rearrange(
                                      "(c p) j -> p c j", p=P))
                        acc = ps2.tile([P, KC], FP)
                        for rc in range(NH):
                            nc.tensor.matmul(
                                acc[:], ab[:, rc, :],
                                mf[:, par * NH + rc, :, :],
                                start=(rc == 0), stop=(rc == NH - 1))
                        ot = opool.tile([P, KC], FP)
                        nc.vector.tensor_copy(ot[:], acc[:])
                        # rows l = 2*(lt*128 + p) + par
                        nc.sync.dma_start(
                            out_d[2 * lt * P + par:2 * (lt + 1) * P:2, :],
                            ot[:])

    nc.compile()
    return nc


def _get_nc(reps=1):
    key = f"nc{reps}"
    if key not in _NC_CACHE:
        _NC_CACHE[key] = _build_nc(reps)
    return _NC_CACHE[key]


def _make_in_maps(x, expk0, expk1):
    return _prep(x, expk0, expk1)


def kernel(x, expk0, expk1):
    from concourse.bass_utils import run_bass_kernel_spmd

    in_maps = _prep(x, expk0, expk1)
    nc = _get_nc()
    res = run_bass_kernel_spmd(nc, in_maps, core_ids=list(range(NCORES)))
    return np.concatenate(
        [res.results[c]["out"] for c in range(NCORES)], axis=1)


if __name__ == "__main__":
    # quick numpy validation of the fold dataflow (dev only; requires the
    # reference module, which is not shipped with this file)
    import jax
    jax.config.update("jax_default_device", jax.devices("cpu")[0])
    import reference

    rng = np.random.default_rng(0)
    inputs = reference.setup_inputs()
    x = np.asarray(inputs["x"])
    e0 = np.asarray(inputs["expk0"])
    e1 = np.asarray(inputs["expk1"])
    expected = np.asarray(reference.reference(**inputs))
    got = _host_sim(x, e0, e1)
    print("host-sim rel err:",
          np.max(np.abs(got - expected)) / np.max(np.abs(expected)))


# revision 8
# speedup vs baseline: 1.1791x; 1.1791x over previous
"""V2: folded direct matmul using the Makhoul reflection symmetry.

For A = build_A(expk):  A[k, N-1-n] = (-1)^k A[k, n]  (holds for arbitrary
expk: it follows from the even/odd permutation structure).  So both stage
contractions fold to length N/2:
  stage1: Mfold[b, k] = sum_{a<H} A1[k, a] * xq[a + H*par(k), b]
  stage2: out[l, k]   = sum_{rf<H} A0[l, rf] * Mfold[b = rf + H*par(l), k]
where xq is x^T folded on both axes (sum-half / diff-half) on the host, and
Mfold rows b<H are the r-sum-folded stage-1 results (b>=H: diff).

Per core: 512 output columns.  PE work per core: 2 * 4.3e9 MACs.
"""
import numpy as np

N = 4096
H = N // 2
P = 128
NT = N // P
KC = 512
NCORES = 8

_NC_CACHE = {}
CHAIN_NAME = "a1w"


def _makhoul_perm(n):
    j = np.arange(n)
    return np.where(j < n // 2, 2 * j, 2 * (n - 1 - j) + 1)


def _build_A(expk, n):
    c = expk[:, 0].astype(np.float64)
    s = expk[:, 1].astype(np.float64)
    k = np.arange(n, dtype=np.int64)
    j = np.arange(n, dtype=np.int64)
    ang = (2.0 * np.pi / n) * ((k[:, None] * j[None, :]) % n).astype(np.float64)
    B = c[:, None] * np.cos(ang) + s[:, None] * np.sin(ang)
    A = np.empty((n, n), dtype=np.float64)
    A[:, _makhoul_perm(n)] = B
    return A.astype(np.float32)


def _fold_rows(m):
    """[N, ...] -> sum-half / diff-half stacked [N, ...]."""
    top, bot = m[:H], m[H:][::-1]
    return np.concatenate([top + bot, top - bot], axis=0)


def _prep(x, expk0, expk1):
    x = np.asarray(x, dtype=np.float32)
    A1 = _build_A(np.asarray(expk1, np.float32), N)
    A0 = _build_A(np.asarray(expk0, np.float32), N)
    xt = np.ascontiguousarray(x.T)                       # [n, r]
    xq = _fold_rows(_fold_rows(xt).T).T                  # fold n (rows) & r (cols)
    xq = np.ascontiguousarray(xq)

    # stage-2 stationary: a0w[:H, j] = A0[2j, rf], a0w[H:, j] = A0[2j+1, rf]
    a0w = np.empty((N, H), dtype=np.float32)
    a0w[:H] = A0[0::2, :H].T
    a0w[H:] = A0[1::2, :H].T
    a0w = np.ascontiguousarray(a0w)

    in_maps = []
    for c in range(NCORES):
        kc = slice(c * KC, (c + 1) * KC)
        A1c = A1[kc]                                     # [512, n]
        a1w = np.empty((N, 256), dtype=np.float32)
        a1w[:H] = A1c[0::2, :H].T                        # even-k weights
        a1w[H:] = A1c[1::2, :H].T                        # odd-k weights
        in_maps.append({"xq": xq, "a1w": np.ascontiguousarray(a1w), "a0w": a0w})
    return in_maps


def _host_sim(x, expk0, expk1):
    """Numpy simulation of the kernel dataflow (for validation)."""
    in_maps = _prep(x, expk0, expk1)
    outs = []
    for c in range(NCORES):
        m = in_maps[c]
        xq, a1w, a0w = m["xq"], m["a1w"], m["a0w"]
        mfold = np.empty((N, KC), dtype=np.float32)
        me = xq[:H].T @ a1w[:H]                          # [b, 256] even k
        mo = xq[H:].T @ a1w[H:]                          # [b, 256] odd k
        mfold[:, 0::2] = me
        mfold[:, 1::2] = mo
        out = np.empty((N, KC), dtype=np.float32)
        out[0::2] = (a0w[:H].T @ mfold[:H])              # even l
        out[1::2] = (a0w[H:].T @ mfold[H:])              # odd l
        outs.append(out)
    return np.concatenate(outs, axis=1)


def _build_nc(reps=1):
    import concourse.bacc as bacc
    import concourse.mybir as mybir
    import concourse.tile as tile

    FP = mybir.dt.float32
    nc = bacc.Bacc("TRN2", target_bir_lowering=False, debug=False,
                   num_devices=NCORES)

    xq_d = nc.dram_tensor("xq", [N, N], FP, kind="ExternalInput")
    a1w_d = nc.dram_tensor("a1w", [N, 256], FP, kind="ExternalInput")
    a0w_d = nc.dram_tensor("a0w", [N, H], FP, kind="ExternalInput")
    out_d = nc.dram_tensor("out", [N, KC], FP, kind="ExternalOutput")

    NH = H // P  # 16 chunks per folded (2048) contraction

    with tile.TileContext(nc) as tc:
      for _rep in range(reps):
        with tc.tile_pool(name="mfold", bufs=1) as mpool:
            # Mfold [b within tile, (b_tile, i, parity)]
            mf = mpool.tile([P, NT, 256, 2], FP)

            with (
                tc.tile_pool(name="a1pool", bufs=1) as a1pool,
                tc.tile_pool(name="xpool", bufs=2) as xpool,
                tc.tile_pool(name="ps1", bufs=4, space="PSUM") as ps1,
            ):
                a1t = a1pool.tile([P, NT, 256], FP)
                nc.sync.dma_start(
                    a1t[:], a1w_d[:].rearrange("(c p) k -> p c k", p=P))

                for bt2 in range(NT // 2):  # 256-column xq blocks (1KB runs)
                    xb = xpool.tile([P, NT, 256], FP)
                    nc.sync.dma_start(
                        xb[:],
                        xq_d[:, bt2 * 256:(bt2 + 1) * 256].rearrange(
                            "(c p) b -> p c b", p=P))
                    for half in range(2):
                        bt = 2 * bt2 + half
                        bsl = slice(half * P, (half + 1) * P)
                        pse = ps1.tile([P, 256], FP)
                        pso = ps1.tile([P, 256], FP)
                        for ac in range(NH):
                            nc.tensor.matmul(
                                pse[:], xb[:, ac, bsl], a1t[:, ac, :],
                                start=(ac == 0), stop=(ac == NH - 1))
                        for ac in range(NH):
                            nc.tensor.matmul(
                                pso[:], xb[:, NH + ac, bsl], a1t[:, NH + ac, :],
                                start=(ac == 0), stop=(ac == NH - 1))
                        nc.vector.tensor_copy(mf[:, bt, :, 0], pse[:])
                        nc.vector.tensor_copy(mf[:, bt, :, 1], pso[:])

            with (
                tc.tile_pool(name="a0pool", bufs=2) as a0pool,
                tc.tile_pool(name="opool", bufs=4) as opool,
                tc.tile_pool(name="ps2", bufs=8, space="PSUM") as ps2,
            ):
                for par in range(2):
                    for lt in range(H // P):  # 16 tiles of 128 j's
                        ab = a0pool.tile([P, NH, P], FP)
                        nc.sync.dma_start(
                            ab[:],
                            a0w_d[par * H:(par + 1) * H,
                                  lt * P:(lt + 1) * P].rearrange(
                                      "(c p) j -> p c j", p=P))
                        acc = ps2.tile([P, KC], FP)
                        for rc in range(NH):
                            nc.tensor.matmul(
                                acc[:], ab[:, rc, :],
                                mf[:, par * NH + rc, :, :],
                                start=(rc == 0), stop=(rc == NH - 1))
                        ot = opool.tile([P, KC], FP)
                        nc.vector.tensor_copy(ot[:], acc[:])
                        # rows l = 2*(lt*128 + p) + par
                        nc.sync.dma_start(
                            out_d[2 * lt * P + par:2 * (lt + 1) * P:2, :],
                            ot[:])

    nc.compile()
    return nc


def _get_nc(reps=1):
    key = f"nc{reps}"
    if key not in _NC_CACHE:
        _NC_CACHE[key] = _build_nc(reps)
    return _NC_CACHE[key]


def _make_in_maps(x, expk0, expk1):
    return _prep(x, expk0, expk1)


def kernel(x, expk0, expk1):
    from concourse.bass_utils import run_bass_kernel_spmd

    in_maps = _prep(x, expk0, expk1)
    nc = _get_nc()
    res = run_bass_kernel_spmd(nc, in_maps, core_ids=list(range(NCORES)))
    return np.concatenate(
        [res.results[c]["out"] for c in range(NCORES)], axis=1)


if __name__ == "__main__":
    # quick numpy validation of the fold dataflow (dev only; requires the
    # reference module, which is not shipped with this file)
    import jax
    jax.config.update("jax_default_device", jax.devices("cpu")[0])
    import reference

    rng = np.random.default_rng(0)
    inputs = reference.setup_inputs()
    x = np.asarray(inputs["x"])
    e0 = np.asarray(inputs["expk0"])
    e1 = np.asarray(inputs["expk1"])
    expected = np.asarray(reference.reference(**inputs))
    got = _host_sim(x, e0, e1)
    print("host-sim rel err:",
          np.max(np.abs(got - expected)) / np.max(np.abs(expected)))
